# Initial kernel scaffold
#
"""Conv2d 3x3 (stride 1, pad 1, cross-correlation) + scalar bias on 8 TRN2 cores.

Full inputs:  x (32, 128, 56, 56) f32, K (256, 128, 3, 3) f32, bias (1,) f32
Full output:  (32, 256, 56, 56) f32

Sharding: data-parallel over the batch dim — each of the 8 NeuronCores gets 4
images; K and bias are replicated. No collectives needed.

Per-core algorithm (implicit GEMM via shifted matmuls):
  - Host zero-pads each image to 58x58 and lays it out as [Cin=128, 58*58]
    (Cin on SBUF partitions = the matmul contraction dim).
  - For each output row-tile of 8 padded rows (8*58 = 464 moving elements) and
    each Cout chunk of 128, accumulate 9 matmuls in one PSUM bank:
        out[co, p] += K[co, ci, dy, dx] * xpad[ci, p + (dy-1)*58 + (dx-1)]
    lhsT = K slice [ci=128, co=128] (stationary), rhs = shifted xpad slice.
  - dtype float32r: fp32 bits in memory, PE runs it at full (bf16) rate for
    moving dims >= 256.
  - PSUM is evacuated through ScalarE activation(Copy, bias=...) which folds in
    the scalar bias, then DMA'd to HBM in a padded 58-wide layout; the host
    strips the 2 padding columns.
"""

import numpy as np

import concourse.tile as tile
import concourse.mybir as mybir
from concourse import bacc
from concourse import bass_utils

N, CIN, H, W = 32, 128, 56, 56
COUT, KH, KW = 256, 3, 3
NCORES = 8
B = N // NCORES            # images per core
HP, WP = H + 2, W + 2      # zero-padded image dims (58x58)
IMG = HP * WP              # 3364
XLEN = IMG + 2             # +1 lead/tail margin so shifted reads stay in-bounds
ROWS = 8                   # output rows per PSUM tile
NT = ROWS * WP             # 464 moving elements per matmul (<= 512 fp32/bank)
R_STARTS = [1 + ROWS * i for i in range(H // ROWS)]  # 7 row-tiles, padded row idx
OWPAD = H * WP             # padded output block per (n, co): 56 rows * 58 cols
GROUP = 4                  # row-tiles sharing one weight load sequence

F32 = mybir.dt.float32
F32R = mybir.dt.float32r

_CACHE = {}


def _build():
    nc = bacc.Bacc("TRN2", target_bir_lowering=False, debug=False)

    x_d = [
        nc.dram_tensor(f"x{n}", [CIN, XLEN], F32R, kind="ExternalInput")
        for n in range(B)
    ]
    kw_d = nc.dram_tensor("kw", [CIN, KH * KW * COUT], F32R, kind="ExternalInput")
    b_d = nc.dram_tensor("biasv", [CIN, 1], F32, kind="ExternalInput")
    y_d = nc.dram_tensor("y", [B, COUT, OWPAD], F32, kind="ExternalOutput")

    with tile.TileContext(nc) as tc:
        with (
            tc.tile_pool(name="const", bufs=1) as const,
            tc.tile_pool(name="psum", bufs=8, space="PSUM") as psum,
            tc.tile_pool(name="outs", bufs=6) as outs,
        ):
            kw = const.tile([CIN, KH * KW * COUT], F32R, tag="kw")
            nc.sync.dma_start(kw[:], kw_d[:])
            bias = const.tile([CIN, 1], F32, tag="bias")
            nc.sync.dma_start(bias[:], b_d[:])
            xin = []
            for n in range(B):
                xt = const.tile([CIN, XLEN], F32R, tag=f"x{n}")
                nc.sync.dma_start(xt[:], x_d[n][:])
                xin.append(xt)

            tiles = [(n, r) for n in range(B) for r in R_STARTS]  # 28 row-tiles
            for g in range(0, len(tiles), GROUP):
                grp = tiles[g : g + GROUP]
                for chunk in range(2):
                    pts = [psum.tile([128, NT], F32) for _ in grp]
                    ki = 0
                    for dy in range(3):
                        for dx in range(3):
                            w = kw[
                                :,
                                (dy * 3 + dx) * COUT + chunk * 128 :
                                (dy * 3 + dx) * COUT + chunk * 128 + 128,
                            ]
                            shift = (dy - 1) * WP + (dx - 1)
                            for t, (n, r) in enumerate(grp):
                                rhs = xin[n][
                                    :, 1 + r * WP + shift : 1 + r * WP + shift + NT
                                ]
                                nc.tensor.matmul(
                                    pts[t][:], w, rhs, start=(ki == 0), stop=(ki == 8)
                                )
                            ki += 1
                    for t, (n, r) in enumerate(grp):
                        ot = outs.tile([128, NT], F32)
                        nc.scalar.activation(
                            ot[:],
                            pts[t][:],
                            mybir.ActivationFunctionType.Copy,
                            bias=bias[:],
                        )
                        nc.sync.dma_start(
                            y_d[
                                n,
                                chunk * 128 : (chunk + 1) * 128,
                                (r - 1) * WP : (r - 1) * WP + NT,
                            ],
                            ot[:],
                        )

    nc.compile()
    return nc


def _get_nc():
    if "nc" not in _CACHE:
        _CACHE["nc"] = _build()
    return _CACHE["nc"]


def _prep_in_maps(x, K, bias):
    x = np.ascontiguousarray(x, dtype=np.float32)
    K = np.ascontiguousarray(K, dtype=np.float32)
    bias = np.asarray(bias, dtype=np.float32)

    # kw[ci, (dy*3+dx)*COUT + co] = K[co, ci, dy, dx]
    kw = np.ascontiguousarray(K.transpose(1, 2, 3, 0).reshape(CIN, KH * KW * COUT))
    biasv = np.full((CIN, 1), bias.reshape(-1)[0], dtype=np.float32)

    # Per-core padded inputs: [CIN, 1 + 58*58 + 1] with zero borders/margins.
    xbuf = np.zeros((NCORES, B, CIN, XLEN), dtype=np.float32)
    view = xbuf[:, :, :, 1 : 1 + IMG].reshape(NCORES, B, CIN, HP, WP)
    view[:, :, :, 1 : 1 + H, 1 : 1 + W] = x.reshape(NCORES, B, CIN, H, W)

    in_maps = []
    for c in range(NCORES):
        m = {"kw": kw, "biasv": biasv}
        for n in range(B):
            m[f"x{n}"] = np.ascontiguousarray(xbuf[c, n])
        in_maps.append(m)
    return in_maps


def run_on_cores(x, K, bias, trace=False):
    """Run the SPMD kernel; returns (full_output, BassKernelResults)."""
    nc = _get_nc()
    in_maps = _prep_in_maps(x, K, bias)
    res = bass_utils.run_bass_kernel_spmd(
        nc, in_maps, core_ids=list(range(NCORES)), trace=trace
    )
    out = np.empty((N, COUT, H, W), dtype=np.float32)
    for c in range(NCORES):
        ypad = res.results[c]["y"].reshape(B, COUT, H, WP)
        out[c * B : (c + 1) * B] = ypad[:, :, :, 1 : 1 + W]
    return out, res


def kernel(x, K, bias):
    out, _ = run_on_cores(x, K, bias, trace=False)
    return out


# revision 6
# speedup vs baseline: 1.0484x; 1.0484x over previous
"""Conv2d 3x3 (stride 1, pad 1, cross-correlation) + scalar bias on 8 TRN2 cores.

Full inputs:  x (32, 128, 56, 56) f32, K (256, 128, 3, 3) f32, bias (1,) f32
Full output:  (32, 256, 56, 56) f32

Sharding: data-parallel over the batch dim — each of the 8 NeuronCores gets 4
images; K and bias are replicated. No collectives needed.

Per-core algorithm (implicit GEMM via shifted matmuls):
  - Host zero-pads each image to 58x58 and lays it out as [Cin=128, 58*58]
    (Cin on SBUF partitions = the matmul contraction dim).
  - For each output row-tile of 8 padded rows (8*58 = 464 moving elements) and
    each Cout chunk of 128, accumulate 9 matmuls in one PSUM bank:
        out[co, p] += K[co, ci, dy, dx] * xpad[ci, p + (dy-1)*58 + (dx-1)]
    lhsT = K slice [ci=128, co=128] (stationary), rhs = shifted xpad slice.
  - dtype float32r: fp32 bits in memory, PE runs it at full (bf16) rate for
    moving dims >= 256.
  - PSUM is evacuated through ScalarE activation(Copy, bias=...) which folds in
    the scalar bias, then DMA'd to HBM in a padded 58-wide layout; the host
    strips the 2 padding columns.
"""

import numpy as np

import concourse.tile as tile
import concourse.mybir as mybir
from concourse import bacc
from concourse import bass_utils

N, CIN, H, W = 32, 128, 56, 56
COUT, KH, KW = 256, 3, 3
NCORES = 8
B = N // NCORES            # images per core
HP, WP = H + 2, W + 2      # zero-padded image dims (58x58)
IMG = HP * WP              # 3364
XLEN = IMG + 2             # +1 lead/tail margin so shifted reads stay in-bounds
ROWS = 8                   # output rows per PSUM tile
NT = ROWS * WP             # 464 moving elements per matmul (<= 512 fp32/bank)
R_STARTS = [1 + ROWS * i for i in range(H // ROWS)]  # 7 row-tiles, padded row idx
OWPAD = H * WP             # padded output block per (n, co): 56 rows * 58 cols
GROUP = 4                  # row-tiles sharing one weight load sequence

F32 = mybir.dt.float32
F32R = mybir.dt.float32r

_CACHE = {}


def _build(nreps=1):
    nc = bacc.Bacc("TRN2", target_bir_lowering=False, debug=False)

    x_d = [
        nc.dram_tensor(f"x{n}", [CIN, XLEN], F32R, kind="ExternalInput")
        for n in range(B)
    ]
    kw_d = nc.dram_tensor("kw", [CIN, KH * KW * COUT], F32R, kind="ExternalInput")
    b_d = nc.dram_tensor("biasv", [CIN, 1], F32, kind="ExternalInput")
    y_d = nc.dram_tensor("y", [B, COUT, OWPAD], F32, kind="ExternalOutput")

    with tile.TileContext(nc) as tc:
        rep_ctx = tc.For_i(0, nreps, 1) if nreps > 1 else None
        if rep_ctx is not None:
            rep_ctx.__enter__()
        with (
            tc.tile_pool(name="const", bufs=1) as const,
            tc.tile_pool(name="psum", bufs=8, space="PSUM") as psum,
            tc.tile_pool(name="outs", bufs=6) as outs,
        ):
            kw = const.tile([CIN, KH * KW * COUT], F32R, tag="kw")
            nc.sync.dma_start(kw[:], kw_d[:])
            bias = const.tile([CIN, 1], F32, tag="bias")
            nc.sync.dma_start(bias[:], b_d[:])
            xin = []
            for n in range(B):
                xt = const.tile([CIN, XLEN], F32R, tag=f"x{n}")
                nc.sync.dma_start(xt[:], x_d[n][:])
                xin.append(xt)

            tiles = [(n, r) for n in range(B) for r in R_STARTS]  # 28 row-tiles
            for g in range(0, len(tiles), GROUP):
                grp = tiles[g : g + GROUP]
                for chunk in range(2):
                    pts = [
                        psum.tile([128, NT], F32, name="pt", tag="pt") for _ in grp
                    ]
                    ki = 0
                    for dy in range(3):
                        for dx in range(3):
                            w = kw[
                                :,
                                (dy * 3 + dx) * COUT + chunk * 128 :
                                (dy * 3 + dx) * COUT + chunk * 128 + 128,
                            ]
                            shift = (dy - 1) * WP + (dx - 1)
                            for t, (n, r) in enumerate(grp):
                                rhs = xin[n][
                                    :, 1 + r * WP + shift : 1 + r * WP + shift + NT
                                ]
                                nc.tensor.matmul(
                                    pts[t][:], w, rhs, start=(ki == 0), stop=(ki == 8)
                                )
                            ki += 1
                    for t, (n, r) in enumerate(grp):
                        ot = outs.tile([128, NT], F32)
                        nc.scalar.activation(
                            ot[:],
                            pts[t][:],
                            mybir.ActivationFunctionType.Identity,
                            bias=bias[:],
                        )
                        nc.sync.dma_start(
                            y_d[
                                n,
                                chunk * 128 : (chunk + 1) * 128,
                                (r - 1) * WP : (r - 1) * WP + NT,
                            ],
                            ot[:],
                        )
        if rep_ctx is not None:
            rep_ctx.__exit__(None, None, None)

    nc.compile()
    return nc


def _get_nc():
    if "nc" not in _CACHE:
        _CACHE["nc"] = _build()
    return _CACHE["nc"]


def _prep_in_maps(x, K, bias):
    x = np.ascontiguousarray(x, dtype=np.float32)
    K = np.ascontiguousarray(K, dtype=np.float32)
    bias = np.asarray(bias, dtype=np.float32)

    # kw[ci, (dy*3+dx)*COUT + co] = K[co, ci, dy, dx]
    kw = np.ascontiguousarray(K.transpose(1, 2, 3, 0).reshape(CIN, KH * KW * COUT))
    biasv = np.full((CIN, 1), bias.reshape(-1)[0], dtype=np.float32)

    # Per-core padded inputs: [CIN, 1 + 58*58 + 1] with zero borders/margins.
    xbuf = np.zeros((NCORES, B, CIN, XLEN), dtype=np.float32)
    view = xbuf[:, :, :, 1 : 1 + IMG].reshape(NCORES, B, CIN, HP, WP)
    view[:, :, :, 1 : 1 + H, 1 : 1 + W] = x.reshape(NCORES, B, CIN, H, W)

    in_maps = []
    for c in range(NCORES):
        m = {"kw": kw, "biasv": biasv}
        for n in range(B):
            m[f"x{n}"] = np.ascontiguousarray(xbuf[c, n])
        in_maps.append(m)
    return in_maps


def run_on_cores(x, K, bias, trace=False):
    """Run the SPMD kernel; returns (full_output, BassKernelResults)."""
    nc = _get_nc()
    in_maps = _prep_in_maps(x, K, bias)
    res = bass_utils.run_bass_kernel_spmd(
        nc, in_maps, core_ids=list(range(NCORES)), trace=trace
    )
    out = np.empty((N, COUT, H, W), dtype=np.float32)
    for c in range(NCORES):
        ypad = res.results[c]["y"].reshape(B, COUT, H, WP)
        out[c * B : (c + 1) * B] = ypad[:, :, :, 1 : 1 + W]
    return out, res


def kernel(x, K, bias):
    out, _ = run_on_cores(x, K, bias, trace=False)
    return out
